# revision 56
# baseline (speedup 1.0000x reference)
"""Multi-head attention (b=2, n=2048, e=1024, h=16, d=64) on 8 trn2 NeuronCores.

Sharding: data-parallel over batch (2) x tensor-parallel over heads (16 -> 4
groups of 4). Core c handles batch c//4, heads 4*(c%4) .. 4*(c%4)+3.
Each core computes the qkv projection for its heads, full attention, and a
row-parallel slice of the output projection; the host sums the 4 partial
projections per batch (bf16 partials) and adds bproj.

v2 design (vs the v1 baseline):
 - head-PAIR processing: the two heads of an hc group live at partitions
   0-63 / 64-127, so their K=64 energy matmuls target disjoint PE row tiles
   (tile_position rows 0/64) and execute concurrently in the systolic array
   (HW probe: 128ns/MM alternating vs 446ns/MM same-tile back-to-back).
 - exp split across engines: head A's exp on ScalarE (table exp), head B's
   on DVE as a one-instruction Schraudolph bit-trick (int16 = e*A + B,
   bitcast bf16; ~2-4% ripple that cancels in the softmax ratio since the
   denominator uses the same approximation).
 - software-pipelined attention rounds: AV matmuls trail energy matmuls by
   2 rounds in the PE stream so the PE never waits on the PSUM->exp->SBUF
   round trip.
 - qkv/v/proj matmuls interleave into attention rounds ("extras");
   normalization muls run on the otherwise-idle Pool (gpsimd) engine;
   PSUM evacuations on ACT; v-bias/recip/schraudolph on DVE.
 - inputs are host-relaid so each tensor arrives in 1-4 large DMAs;
   the recip DRAM-roundtrip broadcast is batched per unit-pair; output
   partials are bf16 and written one DMA per 128-token tile.
"""

import numpy as np
import ml_dtypes

import concourse.bass as bass
import concourse.tile as tile
from concourse import bacc, mybir
from concourse import bass_utils

B, N, E, H, D = 2, 2048, 1024, 16, 64
NCORES = 8
HPC = H // 4  # heads per core = 4
DC = HPC * D  # dcols per core = 256
EC = E // 128  # 8 e-chunks
NT = N // 128  # 16 token tiles
QC = N // 512  # 4 q-chunks of 512
F32 = mybir.dt.float32
BF16 = mybir.dt.bfloat16
I16 = mybir.dt.int16
BF = ml_dtypes.bfloat16

LN2 = float(np.log(2.0))
A16 = (2.0 ** 7) / LN2          # bf16-space schraudolph slope
C16 = 486411.0 / 65536.0        # ripple-centering constant
B16 = 127.0 * 2 ** 7 - C16

_CACHE = {}


def build_nc(debug_outs=False, reps=None, tiny_out=False):
    nc = bacc.Bacc("TRN2", target_bir_lowering=False, debug=False, num_devices=NCORES)

    # host-relaid inputs: partition dim first, e-chunk second
    xT_d = nc.dram_tensor("xT", [128, EC * N], BF16, kind="ExternalInput")
    wq_d = nc.dram_tensor("wq", [128, EC * DC], BF16, kind="ExternalInput")
    wk_d = nc.dram_tensor("wk", [128, EC * DC], BF16, kind="ExternalInput")
    wv_d = nc.dram_tensor("wv", [128, EC * DC], BF16, kind="ExternalInput")
    wp_d = nc.dram_tensor("wp", [128, 2 * E], BF16, kind="ExternalInput")
    bqT_d = nc.dram_tensor("bqT", [128, 2], F32, kind="ExternalInput")
    bkT_d = nc.dram_tensor("bkT", [128, 2], F32, kind="ExternalInput")
    bvb_d = nc.dram_tensor("bvb", [128, DC], F32, kind="ExternalInput")
    out_rows = 512 if tiny_out else N
    out_d = nc.dram_tensor("out", [out_rows, E], BF16, kind="ExternalOutput")
    if debug_outs:
        kT_dbg = nc.dram_tensor("kT_dbg", [128, 2 * N], BF16, kind="ExternalOutput")
        qT_dbg = nc.dram_tensor("qT_dbg", [128, 2 * N], BF16, kind="ExternalOutput")
        v_dbg = nc.dram_tensor("v_dbg", [128, NT * HPC * 66], BF16, kind="ExternalOutput")
        outT_dbg = nc.dram_tensor("outT_dbg", [128, 2 * N], BF16, kind="ExternalOutput")
        av_dbg = nc.dram_tensor("av_dbg", [65, 2 * 512], F32, kind="ExternalOutput")
        rbc_dbg = nc.dram_tensor("rbc_dbg", [64, 2 * 512], F32, kind="ExternalOutput")

    inv_scale = 1.0 / float(np.sqrt(np.float32(E)))

    with tile.TileContext(nc) as tc:
        with (
            tc.tile_pool(name="const", bufs=1) as const,
            tc.tile_pool(name="stg", bufs=4, space="PSUM") as stg_pool,
            tc.tile_pool(name="av", bufs=2, space="PSUM") as av_pool,
            tc.tile_pool(name="io", bufs=2, space="PSUM") as io_pool,
            tc.tile_pool(name="ex", bufs=6) as ex_pool,
            tc.tile_pool(name="nrm", bufs=4) as nrm_pool,
            tc.tile_pool(name="dscr", bufs=4, space="DRAM") as dscr_pool,
            tc.tile_pool(name="outst", bufs=4) as outst_pool,
        ):
            def emit_body():
                # ---- persistent SBUF tensors ----
                xT_sb = const.tile([128, EC, N], BF16)
                wq_sb = const.tile([128, EC, DC], BF16)
                wk_sb = const.tile([128, EC, DC], BF16)
                wv_sb = const.tile([128, EC, DC], BF16)
                wp_sb = const.tile([128, 2, E], BF16)
                bqT_sb = const.tile([128, 2], F32)
                bkT_sb = const.tile([128, 2], F32)
                bvb_sb = const.tile([128, DC], F32)
                qT_sb = const.tile([128, 2, N], BF16)
                kT_sb = const.tile([128, 2, N], BF16)
                # per l-tile, per head: 64 dims + ones col + pad -> lhsT [128, 66]
                v_sb = const.tile([128, NT, HPC, 66], BF16)
                outT_sb = const.tile([128, 2, N], BF16)

                # ---- input DMAs: sync queue carries xT (t-chunk at a time),
                # scalar queue carries weights/biases ----
                xT_ap3 = xT_d.ap().rearrange("p (ec n) -> p ec n", ec=EC)
                nc.sync.dma_start(out=xT_sb[:, :, 0:256], in_=xT_ap3[:, :, 0:256])
                nc.sync.dma_start(out=xT_sb[:, :, 256:512], in_=xT_ap3[:, :, 256:512])
                for t in range(1, QC):
                    nc.sync.dma_start(
                        out=xT_sb[:, :, t * 512:(t + 1) * 512],
                        in_=xT_ap3[:, :, t * 512:(t + 1) * 512],
                    )
                nc.scalar.dma_start(
                    out=wk_sb[:], in_=wk_d.ap().rearrange("p (ec d) -> p ec d", ec=EC))
                nc.scalar.dma_start(out=bkT_sb[:], in_=bkT_d.ap())
                nc.scalar.dma_start(
                    out=wv_sb[:], in_=wv_d.ap().rearrange("p (ec d) -> p ec d", ec=EC))
                nc.scalar.dma_start(out=bvb_sb[:], in_=bvb_d.ap())
                nc.scalar.dma_start(out=bqT_sb[:], in_=bqT_d.ap())
                nc.scalar.dma_start(
                    out=wq_sb[:], in_=wq_d.ap().rearrange("p (ec d) -> p ec d", ec=EC))
                nc.scalar.dma_start(
                    out=wp_sb[:], in_=wp_d.ap().rearrange("p (hc e) -> p hc e", hc=2))

                nc.gpsimd.memset(v_sb[:, :, :, 64:65], 1.0)

                # PE warm-up: keep the array busy while input DMAs stream so
                # the HAM clock gate is at 8/8 when real work arrives
                wu = const.tile([1, 128], BF16)
                nc.vector.memset(wu[:], 1.0)
                for i in range(8):
                    wups = io_pool.tile([128, 512], F32, tag="io", name=f"wu{i}")
                    nc.tensor.matmul(wups[0:128, 0:128], lhsT=wu[:], rhs=wu[:],
                                     start=True, stop=True)

                QK = ((wk_sb, bkT_sb, kT_sb), (wq_sb, bqT_sb, qT_sb))
                qkb_i = [0]

                # ---- extras: generator-based interleave of qkv/v/proj MMs ----
                def gen_qk_group(which, m, t):
                    """yield one PE matmul step at a time; bias-add at end (DVE)."""
                    w_sb, b_sb, dst = QK[which]
                    pq = io_pool.tile([128, 512], F32, tag="io")
                    for ec in range(EC):
                        nc.tensor.matmul(
                            pq[:],
                            lhsT=w_sb[:, ec, m * 128:(m + 1) * 128],
                            rhs=xT_sb[:, ec, t * 512:(t + 1) * 512],
                            start=(ec == 0), stop=(ec == EC - 1),
                        )
                        if ec < EC - 1:
                            yield
                    qkb_i[0] += 1
                    if qkb_i[0] % 2:
                        nc.vector.tensor_scalar_add(
                            out=dst[:, m, t * 512:(t + 1) * 512],
                            in0=pq[:], scalar1=b_sb[:, m:m + 1],
                        )
                    else:
                        nc.scalar.activation(
                            out=dst[:, m, t * 512:(t + 1) * 512], in_=pq[:],
                            func=mybir.ActivationFunctionType.Identity,
                            bias=b_sb[:, m:m + 1], scale=1.0,
                        )
                    yield

                def gen_v_group(lt, pair):
                    """v projection for one l-tile, one head pair (N=128)."""
                    pv = io_pool.tile([128, 512], F32, tag="io")
                    for ec in range(EC):
                        nc.tensor.matmul(
                            pv[:, 0:128],
                            lhsT=xT_sb[:, ec, lt * 128:(lt + 1) * 128],
                            rhs=wv_sb[:, ec, pair * 128:(pair + 1) * 128],
                            start=(ec == 0), stop=(ec == EC - 1),
                        )
                        if ec < EC - 1 and ec % 2 == 1:
                            yield
                    h0 = 2 * pair
                    nc.vector.tensor_add(
                        out=v_sb[:, lt, h0:h0 + 2, 0:64],
                        in0=pv[:, 0:128].rearrange("p (h d) -> p h d", h=2),
                        in1=bvb_sb[:, h0 * 64:(h0 + 2) * 64].rearrange("p (h d) -> p h d", h=2),
                    )
                    yield

                def gen_proj(qt, tail=False):
                    """output projection for one 128-token tile (2 x N=512).
                    Tail variant borrows the idle stg banks (correctly sized
                    [128,512] pool, no readers left) so the four drain
                    generators double-buffer freely instead of fighting over
                    the two io banks."""
                    ot = outst_pool.tile([128, 1024], BF16, tag="ot")
                    for en in range(2):
                        pool, tg = (stg_pool, "stg") if tail else (io_pool, "io")
                        po = pool.tile([128, 512], F32, tag=tg, name=f"po{qt}_{en}")
                        for hcc in range(2):
                            nc.tensor.matmul(
                                po[:],
                                lhsT=outT_sb[:, hcc, qt * 128:(qt + 1) * 128],
                                rhs=wp_sb[:, hcc, en * 512:(en + 1) * 512],
                                start=(hcc == 0), stop=(hcc == 1),
                            )
                            yield
                        nc.scalar.copy(out=ot[:, en * 512:(en + 1) * 512], in_=po[:])
                        yield
                    oq = (qt % 4) if tiny_out else qt
                    (nc.sync if qt % 2 == 0 else nc.scalar).dma_start(
                        out=out_d.ap()[oq * 128:(oq + 1) * 128, :], in_=ot[:])
                    yield

                def chain(*gens):
                    for g in gens:
                        yield from g

                def take(gen, n):
                    """emit up to n steps from gen; return False when done."""
                    if gen is None:
                        return False
                    for _ in range(n):
                        try:
                            next(gen)
                        except StopIteration:
                            return False
                    return True

                # ones row for the PE-based recip broadcast (tail epilogue)
                onesT = const.tile([1, 64], BF16)
                nc.gpsimd.memset(onesT[:], 1.0)

                def gen_epilogue(pair, qc, avA, avB, last=False):
                    """normalization for one unit-pair, one op per step so it
                    staggers across slots. evac ACT, recip DVE, batched DRAM
                    roundtrip broadcast, mulA Pool / mulB DVE. The last unit
                    broadcasts via a K=1 PE matmul instead of DRAM (latency)."""
                    hc = pair
                    qs = slice(qc * 512, (qc + 1) * 512)
                    rc = nrm_pool.tile([1, 1024], F32, tag="recip")
                    den = nrm_pool.tile([1, 1024], F32, tag="den")
                    if last:
                        # short tail chain: no evacuations — normalize muls
                        # read the av accumulators directly from PSUM
                        nc.vector.tensor_copy(out=den[:, 0:512], in_=avA[64:65, :])
                        nc.vector.tensor_copy(out=den[:, 512:1024], in_=avB[64:65, :])
                        nc.vector.reciprocal_approx_fast(out=rc[:], in_=den[:])
                        rc2 = nrm_pool.tile([1, 1024], BF16, tag="rc2")
                        nc.vector.tensor_copy(out=rc2[:], in_=rc[:])
                        rbp = io_pool.tile([128, 512], F32, tag="io")
                        nc.tensor.matmul(rbp[0:64, :], lhsT=onesT[:], rhs=rc2[:, 0:512],
                                         start=True, stop=True)
                        nc.tensor.matmul(rbp[64:128, :], lhsT=onesT[:], rhs=rc2[:, 512:1024],
                                         start=True, stop=True)
                        rbsA = nrm_pool.tile([64, 512], F32, tag="rbsA")
                        rbsB = nrm_pool.tile([64, 512], F32, tag="rbsB")
                        nc.scalar.copy(out=rbsA[:], in_=rbp[0:64, :])
                        nc.scalar.copy(out=rbsB[:], in_=rbp[64:128, :])
                        nc.vector.tensor_mul(out=outT_sb[0:64, hc, qs],
                                             in0=avA[0:64, :], in1=rbsA[:])
                        nc.vector.tensor_mul(out=outT_sb[64:128, hc, qs],
                                             in0=avB[0:64, :], in1=rbsB[:])
                        return
                    avsbA = nrm_pool.tile([65, 512], F32, tag="avsb", bufs=4)
                    nc.scalar.copy(out=avsbA[:], in_=avA[:])
                    yield
                    nc.vector.tensor_copy(out=den[:, 0:512], in_=avsbA[64:65, :])
                    # head B lands at partitions 64-127 so its normalize mul
                    # runs base-aligned on Pool
                    avsbB = nrm_pool.tile([128, 512], F32, tag="avsbB", bufs=2)
                    nc.scalar.copy(out=avsbB[64:128, :], in_=avB[0:64, :])
                    yield
                    nc.vector.tensor_copy(out=den[:, 512:1024], in_=avB[64:65, :])
                    nc.vector.reciprocal_approx_fast(out=rc[:], in_=den[:])
                    yield
                    dscr = dscr_pool.tile([2, 512], F32, tag="dscr")
                    nc.sync.dma_start(out=dscr[:], in_=rc[:])
                    d_ap = dscr[:]
                    bcastA = bass.AP(tensor=d_ap.tensor, offset=d_ap.offset,
                                     ap=[[0, 64], [1, 512]])
                    bcastB = bass.AP(tensor=d_ap.tensor, offset=d_ap.offset + 512,
                                     ap=[[0, 64], [1, 512]])
                    rbcA = nrm_pool.tile([64, 512], F32, tag="rbcA")
                    rbcB = nrm_pool.tile([128, 512], F32, tag="rbcB", bufs=2)
                    nc.sync.dma_start(out=rbcA[:], in_=bcastA)
                    nc.sync.dma_start(out=rbcB[64:128, :], in_=bcastB)
                    yield
                    if debug_outs and pair == 0 and qc == 0:
                        nc.sync.dma_start(out=av_dbg.ap()[:, 0:512], in_=avsbA[:])
                        nc.sync.dma_start(out=av_dbg.ap()[0:64, 512:1024], in_=avsbB[64:128, :])
                        nc.sync.dma_start(out=rbc_dbg.ap()[:, 0:512], in_=rbcA[:])
                        nc.sync.dma_start(out=rbc_dbg.ap()[:, 512:1024], in_=rbcB[64:128, :])
                    nc.gpsimd.tensor_mul(out=outT_sb[0:64, hc, qs],
                                         in0=avsbA[0:64, :], in1=rbcA[:])
                    yield
                    nc.gpsimd.tensor_mul(out=outT_sb[64:128, hc, qs],
                                         in0=avsbB[64:128, :], in1=rbcB[64:128, :])

                # ---- P1 pre-phase: kT pair0 interleaved with v pair0 (each
                # K(t) covers l-tiles 4t..4t+3), then qT pair0 t0 ----
                # K(t0) split into two N=256 half-groups so the PE starts as
                # soon as the first half-width xT chunk lands
                pq0 = io_pool.tile([128, 512], F32, tag="io")
                for half in range(2):
                    cs = slice(half * 256, (half + 1) * 256)
                    for ec in range(EC):
                        nc.tensor.matmul(
                            pq0[:, cs], lhsT=wk_sb[:, ec, 0:128],
                            rhs=xT_sb[:, ec, cs],
                            start=(ec == 0), stop=(ec == EC - 1),
                        )
                nc.vector.tensor_scalar_add(
                    out=kT_sb[:, 0, 0:512], in0=pq0[:], scalar1=bkT_sb[:, 0:1])
                for lt in range(0, 4):
                    for _ in gen_v_group(lt, 0):
                        pass
                for t in range(1, QC):
                    for _ in gen_qk_group(0, 0, t):
                        pass
                    for lt in range(4 * t, 4 * t + 4):
                        for _ in gen_v_group(lt, 0):
                            pass
                for _ in gen_qk_group(1, 0, 0):
                    pass

                # ---- continuous slot machine over all 8 attention units ----
                # unit u = (pair u//4, qc u%4); energy rounds at slot 16u+r,
                # AV lags 2 slots; no drain/fill bubbles between units.
                units = [(u // 4, u % 4) for u in range(8)]
                mm_extras = chain(
                    gen_qk_group(1, 0, 1),
                    gen_qk_group(0, 1, 0),
                    gen_v_group(0, 1), gen_v_group(1, 1),
                    gen_qk_group(1, 1, 0),
                    gen_v_group(2, 1), gen_v_group(3, 1),
                    gen_qk_group(1, 0, 2),
                    gen_qk_group(0, 1, 1),
                    gen_v_group(4, 1), gen_v_group(5, 1),
                    gen_v_group(6, 1), gen_v_group(7, 1),
                    gen_qk_group(1, 0, 3),
                    gen_qk_group(0, 1, 2),
                    gen_v_group(8, 1), gen_v_group(9, 1),
                    gen_v_group(10, 1), gen_v_group(11, 1),
                    gen_qk_group(1, 1, 1),
                    gen_qk_group(0, 1, 3),
                    gen_v_group(12, 1), gen_v_group(13, 1),
                    gen_v_group(14, 1), gen_v_group(15, 1),
                    gen_qk_group(1, 1, 2),
                    gen_qk_group(1, 1, 3),
                )
                pending = []   # (ready_slot, generator) for delayed proj work
                aux_gens = []  # staggered epilogue generators
                av_tiles = {}
                exs = {}
                more_mm = True
                total_slots = 8 * NT + 2
                for s in range(total_slots):
                    u, r = s // NT, s % NT
                    if u < 8:
                        pair, qc = units[u]
                        if r == 0:
                            avA_t = av_pool.tile([65, 512], F32, tag="av", name=f"avA{u}")
                            avB_t = av_pool.tile([65, 512], F32, tag="av", name=f"avB{u}")
                            av_tiles[u] = (avA_t, avB_t)
                        hc = pair
                        qs = slice(qc * 512, (qc + 1) * 512)
                        sA = stg_pool.tile([128, 512], F32, tag="stg")
                        sB = stg_pool.tile([128, 512], F32, tag="stg")
                        ls = slice(r * 128, (r + 1) * 128)
                        nc.tensor.matmul(
                            sA[:], lhsT=kT_sb[0:64, hc, ls],
                            rhs=qT_sb[0:64, hc, qs], start=True, stop=True,
                        )
                        nc.tensor.matmul(
                            sB[:], lhsT=kT_sb[64:128, hc, ls],
                            rhs=qT_sb[64:128, hc, qs], start=True, stop=True,
                        )
                        eA = ex_pool.tile([128, 512], BF16, tag="ex")
                        eB = ex_pool.tile([128, 512], BF16, tag="ex")
                        nc.scalar.activation(
                            out=eA[:], in_=sA[:],
                            func=mybir.ActivationFunctionType.Exp,
                            scale=inv_scale,
                        )
                        nc.vector.tensor_scalar(
                            out=eB[:].bitcast(I16), in0=sB[:],
                            scalar1=A16 * inv_scale, scalar2=B16,
                            op0=mybir.AluOpType.mult, op1=mybir.AluOpType.add,
                        )
                        exs[s] = (eA, eB)
                    s2 = s - 2
                    if s2 >= 0:
                        u2, r2 = s2 // NT, s2 % NT
                        pair2, qc2 = units[u2]
                        eA, eB = exs.pop(s2)
                        avA, avB = av_tiles[u2]
                        nc.tensor.matmul(
                            avA[:], lhsT=v_sb[:, r2, 2 * pair2, 0:65], rhs=eA[:],
                            start=(r2 == 0), stop=(r2 == NT - 1),
                        )
                        nc.tensor.matmul(
                            avB[:], lhsT=v_sb[:, r2, 2 * pair2 + 1, 0:65], rhs=eB[:],
                            start=(r2 == 0), stop=(r2 == NT - 1),
                        )
                        if r2 == NT - 1:
                            aux_gens.append(gen_epilogue(
                                pair2, qc2, avA, avB, last=(u2 == 7)))
                            if pair2 == 1:
                                # proj for this q-chunk, delayed until the
                                # DRAM-roundtrip normalization completes
                                pending.append((s + 12, chain(
                                    *[gen_proj(4 * qc2 + i, tail=(u2 == 7))
                                      for i in range(4)])))
                    # one epilogue op per slot (stagger engine load)
                    if aux_gens and not take(aux_gens[0], 1):
                        aux_gens.pop(0)
                    # extras: matmul filler work
                    for ready, g in list(pending):
                        if s >= ready:
                            pending.remove((ready, g))
                            mm_extras = chain(mm_extras, g) if more_mm else g
                            more_mm = True
                    if more_mm:
                        more_mm = take(mm_extras, 3 if u < 4 else 2)

                # drain: remaining epilogue ops and extras; round-robin the
                # tail proj generators so their matmuls/copies interleave
                while aux_gens:
                    if not take(aux_gens[0], 4):
                        aux_gens.pop(0)
                tail = ([mm_extras] if more_mm else []) + [g for _, g in sorted(pending)]
                while tail:
                    tail = [g for g in tail if take(g, 2)]

                if debug_outs:
                    nc.sync.dma_start(out=kT_dbg.ap(), in_=kT_sb[:].rearrange("p a n -> p (a n)"))
                    nc.sync.dma_start(out=qT_dbg.ap(), in_=qT_sb[:].rearrange("p a n -> p (a n)"))
                    nc.sync.dma_start(out=v_dbg.ap(), in_=v_sb[:].rearrange("p a b c -> p (a b c)"))
                    nc.sync.dma_start(out=outT_dbg.ap(), in_=outT_sb[:].rearrange("p a n -> p (a n)"))

            if reps is None:
                emit_body()
            else:
                with tc.For_i(0, reps, 1, hint_engines=(
                        mybir.EngineType.PE, mybir.EngineType.Activation,
                        mybir.EngineType.DVE, mybir.EngineType.SP)):
                    emit_body()

    nc.compile()
    return nc


def _relay_pmajor(a, chunk):
    """[C*128, X] -> [128, C, X] host re-layout (partition-major)."""
    c = a.shape[0] // 128
    return np.ascontiguousarray(a.reshape(c, 128, -1).transpose(1, 0, 2)).reshape(128, -1)


def make_in_maps(x, Wqkv, bqkv, Wproj):
    W4 = np.ascontiguousarray(Wqkv.reshape(E, H, D, 3))
    b4 = np.ascontiguousarray(bqkv.reshape(H, D, 3))
    in_maps = []
    for c in range(NCORES):
        bi, hg = c // 4, c % 4
        hs = slice(hg * HPC, (hg + 1) * HPC)
        xT = np.ascontiguousarray(x[bi].T).astype(BF)
        wq = W4[:, hs, :, 0].reshape(E, DC).astype(BF)
        wk = W4[:, hs, :, 1].reshape(E, DC).astype(BF)
        wv = W4[:, hs, :, 2].reshape(E, DC).astype(BF)
        wp = Wproj[hg * DC:(hg + 1) * DC, :].astype(BF)
        in_maps.append({
            "xT": _relay_pmajor(xT, EC),
            "wq": _relay_pmajor(wq, EC),
            "wk": _relay_pmajor(wk, EC),
            "wv": _relay_pmajor(wv, EC),
            "wp": _relay_pmajor(wp, 2),
            "bqT": np.ascontiguousarray(b4[hs, :, 0].reshape(2, 128).T).astype(np.float32),
            "bkT": np.ascontiguousarray(b4[hs, :, 1].reshape(2, 128).T).astype(np.float32),
            "bvb": np.ascontiguousarray(np.tile(b4[hs, :, 2].reshape(1, DC), (128, 1))).astype(np.float32),
        })
    return in_maps


def run(inputs, trace=False, **kw):
    if "nc" not in _CACHE:
        _CACHE["nc"] = build_nc()
    nc = _CACHE["nc"]
    in_maps = make_in_maps(inputs["x"], inputs["Wqkv"], inputs["bqkv"], inputs["Wproj"])
    res = bass_utils.run_bass_kernel_spmd(nc, in_maps, core_ids=list(range(NCORES)), trace=trace, **kw)
    out = np.zeros((B, N, E), np.float32)
    for c in range(NCORES):
        out[c // 4] += res.results[c]["out"].astype(np.float32)
    out += inputs["bproj"].astype(np.float32)[None, None, :]
    return out, res


def kernel(**inputs):
    inputs = {k: np.asarray(v) for k, v in inputs.items()}
    out, _ = run(inputs)
    return out.astype(np.float32)
